# revision 3
# baseline (speedup 1.0000x reference)
"""KNN palette-retrieval kernel v2 for Trainium2 (8 NeuronCores, data-parallel).

Same math as the v1 baseline (per-pixel argmax over 24 slot-sims = 21
real palette colors + 3 zero pads; emit the argmax's normalized color; a
1e-20 floor on the max makes all-zero pixels emit 0 exactly; exact-tie
pixels multi-fire, same as v1), restructured around what walrus + HW
measurement actually allow:

  - all 6 sim banks live in ONE wide PSUM tile pa6 [128, 6, nt] (3 banks
    at nt=256, double-buffered), written by 6 fp32r matmuls into its
    nt-col regions; one wide ACT copy drains pa6 -> sims6 (SBUF) so the
    PSUM banks recycle early and the PE stays busy.
  - max chain on DVE (TT/STT are DVE-only on TRN2: walrus NCC_IXCG966
    rejects Pool TensorTensor; SBUF+SBUF operands must share the base
    partition: NCC_IBIR297; two PSUM operands are also rejected):
      t1 = max(sims6[:,0:3,:], sims6[:,3:6,:])   one 3-bank-wide op
      f1 = max(t1_0, t1_1); s = max(f1, t1_2)
      u  = max(s[0:64], copy(s[64:128]))         realign copy (2x mode)
      msb[0:32] = (u[0:32] max 1e-20) max copy(u[32:64])   STT
  - msb replicated to 128 rows by two log-doubling SBUF->SBUF DMAs on
    the sync queue (cheaper than the baseline's 4-cycle/row fp32 mrep
    matmul, and off the vector engines; gpsimd copies measured ~4x the
    cost-model price on HW, so Pool is used for nothing).
  - compares: two 3-bank-wide is_ge ops against msb through a stride-0
    broadcast dim; 6 fp32r mm3 matmuls contract the one-hots into
    pout [96, nt]; ACT copies pout into the output staging buffer.
  - I/O: per-queue DMA is fast (12.6 MB in+out measured ~29 us across
    queues); input rides the sync+gpsimd queues, output scalar+gpsimd.

Engine budget per 256-col tile: DVE 9 ops (~4.2 us, the bottleneck), PE
12 fp32r passes, ACT 2 copies, 4 DMAs.  Measured on HW: ~310 us/rep vs
the v1 baseline's ~430-530 us (TimelineSim agrees: 309 us predicted).

build_nc(reps=N) unrolls the pipeline N times in one NEFF (same weights,
same I/O) for the dispatch-floor-cancelling slope timing in test.py;
kernel() runs reps=1.
"""

import sys

sys.path.insert(0, "/opt/trn_rl_repo")

import numpy as np

B, C, H, W = 16, 3, 512, 512
K = 21
NCORES = 8
BPC = B // NCORES            # batches per core
PXC = BPC * H * W            # pixels per core = 524288
G = 32                       # pixel sets (partition-packed)
REG = PXC // G               # 16384 columns per set
NT = 256                     # pixel columns per compute tile
NTIO = 2048                  # pixel columns per DMA chunk
NB = 6                       # sim banks (6*4 = 24 slots >= 21)

_CACHE: dict = {}


def build_nc(reps=1, nt=NT, regn=REG, ntio=NTIO,
             assign="vvvvv", msb_q="sync", io_q="split",
             bufs=(5, 4, 5, 5), chain_psum=False, mrep=False, yb=2,
             cmp3=True):
    # assign: engines for [t1, f1, s, u, mstt, cmp0, cmp1, cmp2],
    # 'v' = DVE, 'g' = Pool/gpsimd
    key = ("nc2", reps, nt, regn, ntio, assign, msb_q, io_q, bufs,
           chain_psum, mrep, yb, cmp3)
    if key in _CACHE:
        return _CACHE[key]
    from contextlib import ExitStack

    import concourse.tile as tile
    from concourse import bacc, mybir

    f32 = mybir.dt.float32
    f32r = mybir.dt.float32r
    mx = mybir.AluOpType.max
    ge = mybir.AluOpType.is_ge

    nc = bacc.Bacc("TRN2", target_bir_lowering=False, debug=False,
                   num_devices=NCORES)
    x = nc.dram_tensor("x", [C * G, regn], f32r, kind="ExternalInput").ap()
    w1 = nc.dram_tensor("w1", [NB, C * G, 128], f32r,
                        kind="ExternalInput").ap()
    w3 = nc.dram_tensor("w3", [NB, 128, C * G], f32r,
                        kind="ExternalInput").ap()
    w2 = nc.dram_tensor("w2", [G, 128], f32, kind="ExternalInput").ap()
    y = nc.dram_tensor("y", [C * G, regn], f32, kind="ExternalOutput").ap()

    with ExitStack() as ctx:
        tc = ctx.enter_context(tile.TileContext(nc))
        b_s6, b_t, b_m, b_oh = bufs
        wp = ctx.enter_context(tc.tile_pool(name="w", bufs=1))
        inp = ctx.enter_context(tc.tile_pool(name="xin", bufs=2))
        s6p = ctx.enter_context(tc.tile_pool(name="s6", bufs=b_s6))
        sp = ctx.enter_context(tc.tile_pool(name="s", bufs=b_t))
        mp = ctx.enter_context(tc.tile_pool(name="m", bufs=b_m))
        ohp = ctx.enter_context(tc.tile_pool(name="oh", bufs=b_oh))
        yp = ctx.enter_context(tc.tile_pool(name="y", bufs=yb))
        pa_bufs = 2 if nt <= 256 else 1
        pap = ctx.enter_context(
            tc.tile_pool(name="pa", bufs=pa_bufs, space="PSUM"))
        pop = ctx.enter_context(tc.tile_pool(name="po", bufs=2, space="PSUM"))

        w1s, w3s = [], []
        for i in range(NB):
            w1t = wp.tile([C * G, 128], f32r, name=f"w1s{i}")
            nc.sync.dma_start(w1t[:], w1[i])
            w1s.append(w1t)
            w3t = wp.tile([128, C * G], f32r, name=f"w3s{i}")
            nc.sync.dma_start(w3t[:], w3[i])
            w3s.append(w3t)
        if mrep:
            w2s = wp.tile([G, 128], f32)
            nc.sync.dma_start(w2s[:], w2[:])

        msb_eng = getattr(nc, msb_q) if msb_q != "copy" else None
        io_eng = getattr(nc, io_q) if io_q != "split" else None
        # TensorTensor/STT are DVE-only on TRN2 (walrus NCC_IXCG966); Pool
        # (gpsimd) can run copies / tensor_scalar.  assign picks the engine
        # for the five COPY ops [sv, uv, rep1, rep2, rep3]: v=DVE, g=Pool.
        def _copier(ch):
            if ch == "a":
                return nc.scalar.copy
            if ch == "d":
                return lambda out, in_: nc.sync.dma_start(out, in_)
            return (nc.vector if ch == "v" else nc.gpsimd).tensor_copy

        e_sv, e_uv, e_r1, e_r2, e_r3 = [_copier(ch) for ch in assign[:5]]

        tpc = ntio // nt           # tiles per DMA chunk
        ntile = (regn // ntio) * tpc

        def phase_a(t, xin_yout):
            """mm1 + max chain + msb replication for tile t."""
            xin, yout = xin_yout
            xs = xin[:, (t % tpc) * nt:(t % tpc + 1) * nt]
            pa6 = pap.tile([128, 6, nt], f32, tag="pa6", name="pa6")
            for p in range(6):
                nc.tensor.matmul(pa6[:, p, :], w1s[p][:], xs,
                                 start=True, stop=True)
            # ACT drains all six banks to SBUF in one wide copy so pa6
            # frees early and the PE never waits on the chain
            sims6 = s6p.tile([128, 6, nt], f32, tag="sims6")
            nc.scalar.copy(sims6[:], pa6[:])
            t1 = sp.tile([128, 3, nt], f32, tag="t1")
            nc.vector.tensor_tensor(t1[:], sims6[:, 0:3, :],
                                    sims6[:, 3:6, :], mx)
            f1 = sp.tile([128, nt], f32, tag="f1")
            nc.vector.tensor_tensor(f1[:], t1[:, 0, :], t1[:, 1, :], mx)
            s = sp.tile([128, nt], f32, tag="sx")
            nc.vector.tensor_tensor(s[:], f1[:], t1[:, 2, :], mx)
            # partition folds: SBUF+SBUF TT operands must share the base
            # partition (walrus NCC_IBIR297), so realign with cheap copies
            # (Pool can run copies; TT cannot leave DVE)
            sv = sp.tile([64, nt], f32, tag="sv")
            e_sv(sv[:], s[64:128, :])
            u = mp.tile([64, nt], f32, tag="u")
            nc.vector.tensor_tensor(u[:], s[0:64, :], sv[:], mx)
            uv = mp.tile([32, nt], f32, tag="uv")
            e_uv(uv[:], u[32:64, :])
            # m3 lands in msb[0:32] (floor fused); replicate to 128 rows
            # via partition-offset copies (or log-doubling DMAs)
            msb = mp.tile([128, nt], f32, tag="msb")
            nc.vector.scalar_tensor_tensor(msb[0:32, :], u[0:32, :], 1e-20,
                                           uv[:], mx, mx)
            if msb_q == "copy":
                e_r1(msb[32:64, :], msb[0:32, :])
                e_r2(msb[64:96, :], msb[0:32, :])
                e_r3(msb[96:128, :], msb[0:32, :])
            else:
                msb_eng.dma_start(msb[32:64, :], msb[0:32, :])
                msb_eng.dma_start(msb[64:128, :], msb[0:64, :])
            return sims6, msb, yout

        def phase_b(t, st):
            """compares + color matmuls + yout copy for tile t."""
            sims6, msb, yout = st
            pa_pair = sims6[:].rearrange("P (a b) n -> P b a n", a=2)
            msb_bc = msb[:].unsqueeze(1).broadcast_to((128, 2, nt))
            ohs = []
            if cmp3:
                # two triple-wide compares over banks 0-2 / 3-5
                msb_bc3 = msb[:].unsqueeze(1).broadcast_to((128, 3, nt))
                for j in range(2):
                    oh = ohp.tile([128, 3, nt], f32r, tag=f"oh{j}",
                                  name=f"oh{j}")
                    nc.vector.tensor_tensor(oh[:],
                                            sims6[:, 3 * j:3 * j + 3, :],
                                            msb_bc3, ge)
                    ohs.append(oh)
                pout = pop.tile([C * G, nt], f32, tag="po", name="po")
                for mmi in range(6):
                    nc.tensor.matmul(pout[:], w3s[mmi][:],
                                     ohs[mmi // 3][:, mmi % 3, :],
                                     start=(mmi == 0), stop=(mmi == 5))
            else:
                for p in range(3):
                    oh = ohp.tile([128, 2, nt], f32r, tag=f"oh{p}",
                                  name=f"oh{p}")
                    nc.vector.tensor_tensor(oh[:], pa_pair[:, p], msb_bc, ge)
                    ohs.append(oh)
                pout = pop.tile([C * G, nt], f32, tag="po", name="po")
                mmi = 0
                for j in range(2):
                    for p in range(3):
                        nc.tensor.matmul(pout[:], w3s[p + 3 * j][:],
                                         ohs[p][:, j, :],
                                         start=(mmi == 0), stop=(mmi == 5))
                        mmi += 1
            nc.scalar.copy(yout[:, (t % tpc) * nt:(t % tpc + 1) * nt],
                           pout[:])
            if t % tpc == tpc - 1:
                i0 = (t // tpc) * ntio
                if io_q == "split":
                    # 2/3 on the ACT HWDGE queue, 1/3 on the Pool SWDGE
                    # queue: per-queue DMA bandwidth is ~22.5 GB/s, so
                    # spreading I/O over 3 queues (with the input's) is the
                    # difference between 530us and ~190us of I/O time
                    c = 2 * ntio // 3
                    nc.scalar.dma_start(y[:, i0:i0 + c], yout[:, 0:c])
                    nc.gpsimd.dma_start(y[:, i0 + c:i0 + ntio],
                                        yout[:, c:ntio])
                else:
                    io_eng.dma_start(y[:, i0:i0 + ntio], yout[:])

        for r in range(reps):
            pend = None            # (t, state) awaiting phase_b
            chunk = None
            for t in range(ntile):
                if t % tpc == 0:
                    i0 = (t // tpc) * ntio
                    xin = inp.tile([C * G, ntio], f32r, tag="xin")
                    if io_q == "split":
                        c = 2 * ntio // 3
                        nc.sync.dma_start(xin[:, 0:c], x[:, i0:i0 + c])
                        nc.gpsimd.dma_start(xin[:, c:ntio],
                                            x[:, i0 + c:i0 + ntio])
                    else:
                        io_eng.dma_start(xin[:], x[:, i0:i0 + ntio])
                    yout = yp.tile([C * G, ntio], f32, tag="yout")
                    chunk = (xin, yout)
                st = phase_a(t, chunk)
                if pend is not None:
                    phase_b(*pend)
                pend = (t, st)
            phase_b(*pend)

    nc.compile()
    _CACHE[key] = nc
    return nc


def _weights(colors: np.ndarray):
    cn = (colors.astype(np.float64)
          / np.linalg.norm(colors.astype(np.float64), axis=-1, keepdims=True))
    W1 = np.zeros((NB, C * G, 128), np.float32)
    W3 = np.zeros((NB, 128, C * G), np.float32)
    W2 = np.zeros((G, 128), np.float32)
    for i in range(NB):
        for kp in range(4):
            k = 4 * i + kp
            if k >= K:
                continue
            for g in range(G):
                for c in range(C):
                    W1[i, G * c + g, G * kp + g] = cn[k, c]
                    W3[i, G * kp + g, G * c + g] = cn[k, c]
    for g in range(G):
        for kp in range(4):
            W2[g, G * kp + g] = 1.0
    return W1, W2, W3


def stage_inputs(rgb_mask: np.ndarray, colors: np.ndarray):
    W1, W2, W3 = _weights(np.asarray(colors, np.float32))
    in_maps = []
    for i in range(NCORES):
        xc = np.asarray(rgb_mask[BPC * i:BPC * (i + 1)], np.float32)
        xc = np.transpose(xc, (1, 0, 2, 3)).reshape(C * G, REG)
        in_maps.append({
            "x": np.ascontiguousarray(xc),
            "w1": W1, "w2": W2, "w3": W3,
        })
    return in_maps


def gather_outputs(results):
    outs = []
    for i in range(NCORES):
        yb = results[i]["y"].reshape(C, BPC, H, W)
        outs.append(np.transpose(yb, (1, 0, 2, 3)))
    return np.ascontiguousarray(np.concatenate(outs, axis=0))


def run(rgb_mask, colors, trace=False, **kw):
    from concourse.bass_utils import run_bass_kernel_spmd

    nc = build_nc()
    in_maps = stage_inputs(rgb_mask, colors)
    res = run_bass_kernel_spmd(nc, in_maps, core_ids=list(range(NCORES)),
                               trace=trace, **kw)
    return gather_outputs(res.results), res


def kernel(rgb_mask, colors):
    out, _ = run(rgb_mask, colors)
    return out
